# revision 29
# baseline (speedup 1.0000x reference)
"""TRN2 Bass kernel for AdjointODE forward (Euler integration of an MLP vector
field): h_{t+1} = h_t + dt_t * (tanh(h_t @ W1 + b1) @ W2 + b2), 50 steps.

Data-parallel over 8 NeuronCores (batch 32768 -> 4096 rows/core). Per core the
state lives transposed in SBUF as hT [dim=128 partitions, 4096 rows] split
into 8 chunks of 512 (one PSUM bank per fp32 matmul output), grouped in pairs.

Per step and chunk: PE runs layer1 (2 fp16 matmuls) into a 2-bank psum tile,
ACT applies one wide tanh [128,1024] PSUM->SBUF writing fp16 (rounding is free
on ACT), PE runs layer2 (2 accumulating fp16 matmuls, with dt pre-folded into
W2 per step on the host), DVE adds the fp32 increment into the fp32 master h.

The next step's layer1 needs h in fp16. Dedicated converts (CAST /
tensor_scalar) are microcoded 4-12x slow on DVE/GPSIMD, but tensor_tensor with
a PSUM input converts at full rate - so for two chunks per step DVE emits the
mirror as (old h + increment -> fp16) before the fp32 update, and for the
remaining chunks GPSIMD emits (updated h + 0 -> fp16), splitting the mirror
work so no engine saturates.

Numerics: h master fp32, PSUM accumulation fp32; fp16 only on matmul operands
(weights rounded on the host, activations rounded by ACT, h-mirror rounded by
the update op). Measured end-to-end: max-abs error / output-scale = 1.0e-4
vs the fp64 reference (fp32-exact variant of the same schedule: 5.2e-5).

Instructions are emitted in an explicit software-pipeline order over pair-
slots (layer2/update trail layer1/tanh by one pair-slot) so all four engines
stream concurrently; measured HW exec ~427 us with ACT (tanh, the per-element
floor) and PE both ~93% busy.
"""

import numpy as np

import concourse.bacc as bacc
import concourse.tile as tile
from concourse import mybir
from concourse.bass_utils import run_bass_kernel_spmd

F32 = mybir.dt.float32
F16 = mybir.dt.float16
AF = mybir.ActivationFunctionType
ALU = mybir.AluOpType

N_CORES = 8
BATCH, DIM, HID = 32768, 128, 256
ROWS = BATCH // N_CORES  # 4096
CH = 512                 # rows per chunk == one PSUM bank of fp32
NCH = ROWS // CH         # 8 chunks -> 4 pairs
NPAIR = NCH // 2

_cache: dict = {}


def _build(steps: int, b1_zero: bool, b2_zero: bool):
    nc = bacc.Bacc("TRN2", target_bir_lowering=False, debug=False)

    HT = nc.dram_tensor("hT", [DIM, ROWS], F32, kind="ExternalInput")
    W1D = nc.dram_tensor("W1B", [DIM, HID], F16, kind="ExternalInput")
    # per-step dt-scaled W2, packed [steps, 128, 256] fp16 (K-halves side by side)
    W2D = nc.dram_tensor("W2S", [steps, DIM, HID], F16, kind="ExternalInput")
    B1D = nc.dram_tensor("B1P", [DIM, 2], F32, kind="ExternalInput")
    DB2 = nc.dram_tensor("DTB2", [DIM, steps], F32, kind="ExternalInput")
    OUT = nc.dram_tensor("hT_out", [DIM, ROWS], F32, kind="ExternalOutput")

    with tile.TileContext(nc) as tc:
        with (
            tc.tile_pool(name="const", bufs=1) as const,
            tc.tile_pool(name="hp", bufs=1) as hp,
            tc.tile_pool(name="hbp", bufs=1) as hbp,
            tc.tile_pool(name="w2sp", bufs=3) as w2sp,
            tc.tile_pool(name="atp", bufs=6) as atp,
            tc.tile_pool(name="ps1", bufs=3, space="PSUM") as ps1,
            tc.tile_pool(name="ps2", bufs=2, space="PSUM") as ps2,
        ):
            w1 = const.tile([DIM, HID], F16, tag="w1")
            b1t = const.tile([DIM, 2], F32, tag="b1t")
            db2 = const.tile([DIM, steps], F32, tag="db2")
            zero = const.tile([DIM, 2 * CH], F32, tag="zero")
            nc.gpsimd.memset(zero[:], 0.0)
            nc.sync.dma_start(w1[:], W1D[:])

            hs, hbs = [], []
            for p in range(NPAIR):
                h = hp.tile([DIM, 2 * CH], F32, tag=f"h{p}")
                nc.sync.dma_start(h[:], HT[:, p * 2 * CH:(p + 1) * 2 * CH])
                hb = hbp.tile([DIM, 2 * CH], F16, tag=f"hb{p}")
                if p == 0:
                    # split the first pair across engines so chunk 0 is ready
                    # as early as possible
                    nc.vector.tensor_add(hb[:, 0:CH], h[:, 0:CH], zero[:, 0:CH])
                    nc.gpsimd.tensor_add(hb[:, CH:2 * CH], h[:, CH:2 * CH],
                                         zero[:, CH:2 * CH])
                elif p == 1:
                    nc.vector.tensor_add(hb[:, 0:CH], h[:, 0:CH], zero[:, 0:CH])
                    nc.vector.tensor_add(hb[:, CH:2 * CH], h[:, CH:2 * CH],
                                         zero[:, CH:2 * CH])
                else:
                    nc.gpsimd.tensor_add(hb[:], h[:], zero[:])
                hs.append(h)
                hbs.append(hb)
            nc.sync.dma_start(b1t[:], B1D[:])
            nc.sync.dma_start(db2[:], DB2[:])

            def hb_slice(c):
                return hbs[c // 2][:, (c % 2) * CH:(c % 2) * CH + CH]

            n_slots = steps * NPAIR
            w2s_tiles = [None, None]
            ats = {}

            def front(s):
                """layer1 + tanh for pair-slot s; per-step W2S prefetch."""
                t, j = divmod(s, NPAIR)
                if j == 0:
                    w2s = w2sp.tile([DIM, HID], F16, tag="w2s")
                    nc.sync.dma_start(w2s[:], W2D[t])
                    w2s_tiles[t % 2] = w2s
                c0, c1 = 2 * j, 2 * j + 1
                p1a = ps1.tile([DIM, 2 * CH], F32, tag="p1")
                p1b = ps1.tile([DIM, 2 * CH], F32, tag="p1")

                def tanh(c, p1):
                    at = atp.tile([DIM, 2 * CH], F16, tag="at", name=f"at{s}_{c}")
                    if b1_zero:
                        nc.scalar.activation(at[:], p1[:], AF.Tanh)
                    else:
                        nc.scalar.activation(at[:, 0:CH], p1[:, 0:CH], AF.Tanh,
                                             bias=b1t[:, 0:1])
                        nc.scalar.activation(at[:, CH:2 * CH], p1[:, CH:2 * CH],
                                             AF.Tanh, bias=b1t[:, 1:2])
                    ats[c] = at

                nc.tensor.matmul(p1a[:, 0:CH], w1[:, 0:DIM], hb_slice(c0),
                                 start=True, stop=True)
                nc.tensor.matmul(p1b[:, 0:CH], w1[:, 0:DIM], hb_slice(c1),
                                 start=True, stop=True)
                nc.tensor.matmul(p1a[:, CH:2 * CH], w1[:, DIM:HID], hb_slice(c0),
                                 start=True, stop=True)
                tanh(c0, p1a)
                nc.tensor.matmul(p1b[:, CH:2 * CH], w1[:, DIM:HID], hb_slice(c1),
                                 start=True, stop=True)
                tanh(c1, p1b)

            def back(s):
                """layer2 + h update + fp16 mirror for pair-slot s."""
                t, j = divmod(s, NPAIR)
                c0, c1 = 2 * j, 2 * j + 1
                at0 = ats.pop(c0)
                at1 = ats.pop(c1)
                w2s = w2s_tiles[t % 2]
                p2a = ps2.tile([DIM, CH], F32, tag="p2", name=f"p2a{s}")
                p2b = ps2.tile([DIM, CH], F32, tag="p2", name=f"p2b{s}")
                nc.tensor.matmul(p2a[:], w2s[:, 0:DIM], at0[:, 0:CH],
                                 start=True, stop=False, skip_group_check=True)
                nc.tensor.matmul(p2b[:], w2s[:, 0:DIM], at1[:, 0:CH],
                                 start=True, stop=False, skip_group_check=True)
                nc.tensor.matmul(p2a[:], w2s[:, DIM:HID], at0[:, CH:2 * CH],
                                 start=False, stop=True, skip_group_check=True)
                nc.tensor.matmul(p2b[:], w2s[:, DIM:HID], at1[:, CH:2 * CH],
                                 start=False, stop=True, skip_group_check=True)
                h = hs[j]
                hb = hbs[j]
                last = t == steps - 1

                def upd_f32(hsl, p2c):
                    """fp32 master update: h += p2 (+ dt*b2)."""
                    if b2_zero:
                        nc.vector.tensor_add(hsl, hsl, p2c[:])
                    else:
                        nc.vector.scalar_tensor_tensor(
                            hsl, p2c[:], db2[:, t:t + 1], hsl,
                            op0=ALU.add, op1=ALU.add)

                def mirror_dve(hbsl, hsl, p2c):
                    """fp16 mirror from (old h + increment); must run before
                    upd_f32 overwrites h. PSUM input keeps it full-rate."""
                    if b2_zero:
                        nc.vector.tensor_add(hbsl, hsl, p2c[:])
                    else:
                        nc.vector.scalar_tensor_tensor(
                            hbsl, p2c[:], db2[:, t:t + 1], hsl,
                            op0=ALU.add, op1=ALU.add)

                for ci, p2c in ((0, p2a), (1, p2b)):
                    hsl = h[:, ci * CH:(ci + 1) * CH]
                    hbsl = hb[:, ci * CH:(ci + 1) * CH]
                    if j < 2 and ci == 0:
                        # chunks 0 and 2: DVE mirror (pre-update), then update
                        if not last:
                            mirror_dve(hbsl, hsl, p2c)
                        upd_f32(hsl, p2c)
                    elif j < 2:
                        # chunks 1 and 3: update, then GPSIMD chunk mirror
                        upd_f32(hsl, p2c)
                        if not last:
                            nc.gpsimd.tensor_add(hbsl, hsl, zero[:, 0:CH])
                    else:
                        # pairs 2,3: update both chunks, one GPSIMD pair mirror
                        upd_f32(hsl, p2c)
                        if ci == 1 and not last:
                            nc.gpsimd.tensor_add(hb[:], h[:], zero[:])
                if last:
                    q = nc.sync if j < 2 else nc.gpsimd
                    q.dma_start(OUT[:, j * 2 * CH:(j + 1) * 2 * CH], h[:])

            for s in range(n_slots + 1):
                if s < n_slots:
                    front(s)
                if s >= 1:
                    back(s - 1)

    nc.compile()
    return nc


def make_in_maps(inputs_dict):
    """Shard + lay out the full problem inputs into per-core input maps."""
    inputs = np.ascontiguousarray(inputs_dict["inputs"], dtype=np.float32)
    timestamps = np.asarray(inputs_dict["timestamps"], dtype=np.float32)
    W1 = np.asarray(inputs_dict["W1"], dtype=np.float32)
    b1 = np.asarray(inputs_dict["b1"], dtype=np.float32)
    W2 = np.asarray(inputs_dict["W2"], dtype=np.float32)
    b2 = np.asarray(inputs_dict["b2"], dtype=np.float32)

    steps = timestamps.shape[0] - 1
    dts = np.diff(timestamps).astype(np.float32)
    w1b = np.ascontiguousarray(W1).astype(np.float16)
    # [steps, 128, 256]: per-step dt*W2, K-halves packed side by side
    w2pack = np.concatenate([W2[:DIM, :], W2[DIM:, :]], axis=1)  # [128, 256]
    w2s = (dts[:, None, None] * w2pack[None, :, :]).astype(np.float16)
    w2s = np.ascontiguousarray(w2s)
    b1p = np.ascontiguousarray(np.stack([b1[:DIM], b1[DIM:]], axis=1))
    db2 = np.ascontiguousarray(b2[:, None] * dts[None, :]).astype(np.float32)

    in_maps = []
    for i in range(N_CORES):
        shard = inputs[i * ROWS:(i + 1) * ROWS, :]
        in_maps.append({
            "hT": np.ascontiguousarray(shard.T), "W1B": w1b, "W2S": w2s,
            "B1P": b1p, "DTB2": db2,
        })
    return in_maps


def kernel(inputs, timestamps, W1, b1, W2, b2):
    timestamps = np.asarray(timestamps, dtype=np.float32)
    b1 = np.asarray(b1, dtype=np.float32)
    b2 = np.asarray(b2, dtype=np.float32)

    steps = timestamps.shape[0] - 1
    b1_zero = bool(np.all(b1 == 0.0))
    b2_zero = bool(np.all(b2 == 0.0))

    key = (steps, b1_zero, b2_zero)
    if key not in _cache:
        _cache[key] = _build(steps, b1_zero, b2_zero)
    nc = _cache[key]

    in_maps = make_in_maps({
        "inputs": inputs, "timestamps": timestamps, "W1": W1, "b1": b1,
        "W2": W2, "b2": b2,
    })

    # The axon-tunneled device occasionally reports a transient
    # "unrecoverable" state right after an unclean process exit; it clears
    # after a short wait, so retry rather than fail the whole run.
    last_exc = None
    for attempt in range(3):
        try:
            res = run_bass_kernel_spmd(nc, in_maps, core_ids=list(range(N_CORES)))
            break
        except Exception as e:
            last_exc = e
            import time as _time
            _time.sleep(20 * (attempt + 1))
    else:
        raise last_exc

    out = np.empty((BATCH, DIM), dtype=np.float32)
    for i in range(N_CORES):
        out[i * ROWS:(i + 1) * ROWS, :] = res.results[i]["hT_out"].T
    return out


# revision 30
# speedup vs baseline: 1.1952x; 1.1952x over previous
"""TRN2 Bass kernel for AdjointODE forward (Euler integration of an MLP vector
field): h_{t+1} = h_t + dt_t * (tanh(h_t @ W1 + b1) @ W2 + b2), 50 steps.

Data-parallel over 8 NeuronCores (batch 32768 -> 4096 rows/core). Per core the
state lives transposed in SBUF as hT [dim=128 partitions, 4096 rows] split
into 8 chunks of 512 (one PSUM bank per fp32 matmul output), grouped in pairs.

Per step and chunk: PE runs layer1 (2 fp16 matmuls) into a 2-bank psum tile,
ACT applies one wide tanh [128,1024] PSUM->SBUF writing fp16 (rounding is free
on ACT), PE runs layer2 (2 accumulating fp16 matmuls, with dt pre-folded into
W2 per step on the host), DVE adds the fp32 increment into the fp32 master h.

The next step's layer1 needs h in fp16. Dedicated converts (CAST /
tensor_scalar) are microcoded 4-12x slow on DVE/GPSIMD, but tensor_tensor with
a PSUM input converts at full rate - so for two chunks per step DVE emits the
mirror as (old h + increment -> fp16) before the fp32 update, and for the
remaining chunks GPSIMD emits (updated h + 0 -> fp16), splitting the mirror
work so no engine saturates.

Numerics: h master fp32, PSUM accumulation fp32; fp16 only on matmul operands
(weights rounded on the host, activations rounded by ACT, h-mirror rounded by
the update op). Measured end-to-end: max-abs error / output-scale = 1.0e-4
vs the fp64 reference (fp32-exact variant of the same schedule: 5.2e-5).

Instructions are emitted in an explicit software-pipeline order over pair-
slots (layer2/update trail layer1/tanh by one pair-slot) so all four engines
stream concurrently; measured HW exec ~427 us with ACT (tanh, the per-element
floor) and PE both ~93% busy.
"""

import numpy as np

import concourse.bacc as bacc
import concourse.tile as tile
from concourse import mybir
from concourse.bass_utils import run_bass_kernel_spmd

F32 = mybir.dt.float32
F16 = mybir.dt.float16
AF = mybir.ActivationFunctionType
ALU = mybir.AluOpType

N_CORES = 8
BATCH, DIM, HID = 32768, 128, 256
ROWS = BATCH // N_CORES  # 4096
CH = 512                 # rows per chunk == one PSUM bank of fp32
NCH = ROWS // CH         # 8 chunks -> 4 pairs
NPAIR = NCH // 2

_cache: dict = {}


def _build(steps: int, b1_zero: bool, b2_zero: bool):
    nc = bacc.Bacc("TRN2", target_bir_lowering=False, debug=False)

    HT = nc.dram_tensor("hT", [DIM, ROWS], F32, kind="ExternalInput")
    W1D = nc.dram_tensor("W1B", [DIM, HID], F16, kind="ExternalInput")
    # per-step dt-scaled W2, packed [steps, 128, 256] fp16 (K-halves side by side)
    W2D = nc.dram_tensor("W2S", [steps, DIM, HID], F16, kind="ExternalInput")
    B1D = nc.dram_tensor("B1P", [DIM, 2], F32, kind="ExternalInput")
    DB2 = nc.dram_tensor("DTB2", [DIM, steps], F32, kind="ExternalInput")
    OUT = nc.dram_tensor("hT_out", [DIM, ROWS], F32, kind="ExternalOutput")

    with tile.TileContext(nc) as tc:
        with (
            tc.tile_pool(name="const", bufs=1) as const,
            tc.tile_pool(name="hp", bufs=1) as hp,
            tc.tile_pool(name="hbp", bufs=1) as hbp,
            tc.tile_pool(name="w2sp", bufs=3) as w2sp,
            tc.tile_pool(name="atp", bufs=6) as atp,
            tc.tile_pool(name="ps1", bufs=3, space="PSUM") as ps1,
            tc.tile_pool(name="ps2", bufs=2, space="PSUM") as ps2,
        ):
            w1 = const.tile([DIM, HID], F16, tag="w1")
            b1t = const.tile([DIM, 2], F32, tag="b1t")
            db2 = const.tile([DIM, steps], F32, tag="db2")
            zero = const.tile([DIM, 2 * CH], F32, tag="zero")
            nc.gpsimd.memset(zero[:], 0.0)
            nc.sync.dma_start(w1[:], W1D[:])

            hs, hbs = [], []
            for p in range(NPAIR):
                h = hp.tile([DIM, 2 * CH], F32, tag=f"h{p}")
                nc.sync.dma_start(h[:], HT[:, p * 2 * CH:(p + 1) * 2 * CH])
                hb = hbp.tile([DIM, 2 * CH], F16, tag=f"hb{p}")
                if p < 2:
                    nc.vector.tensor_add(hb[:, 0:CH], h[:, 0:CH], zero[:, 0:CH])
                    nc.vector.tensor_add(hb[:, CH:2 * CH], h[:, CH:2 * CH],
                                         zero[:, CH:2 * CH])
                else:
                    nc.gpsimd.tensor_add(hb[:], h[:], zero[:])
                hs.append(h)
                hbs.append(hb)
            nc.sync.dma_start(b1t[:], B1D[:])
            nc.sync.dma_start(db2[:], DB2[:])

            def hb_slice(c):
                return hbs[c // 2][:, (c % 2) * CH:(c % 2) * CH + CH]

            n_slots = steps * NPAIR
            w2s_tiles = [None, None]
            ats = {}

            def front(s):
                """layer1 + tanh for pair-slot s; per-step W2S prefetch."""
                t, j = divmod(s, NPAIR)
                if j == 0:
                    w2s = w2sp.tile([DIM, HID], F16, tag="w2s")
                    nc.sync.dma_start(w2s[:], W2D[t])
                    w2s_tiles[t % 2] = w2s
                c0, c1 = 2 * j, 2 * j + 1
                p1a = ps1.tile([DIM, 2 * CH], F32, tag="p1")
                p1b = ps1.tile([DIM, 2 * CH], F32, tag="p1")

                def tanh(c, p1):
                    at = atp.tile([DIM, 2 * CH], F16, tag="at", name=f"at{s}_{c}")
                    if b1_zero:
                        nc.scalar.activation(at[:], p1[:], AF.Tanh)
                    else:
                        nc.scalar.activation(at[:, 0:CH], p1[:, 0:CH], AF.Tanh,
                                             bias=b1t[:, 0:1])
                        nc.scalar.activation(at[:, CH:2 * CH], p1[:, CH:2 * CH],
                                             AF.Tanh, bias=b1t[:, 1:2])
                    ats[c] = at

                nc.tensor.matmul(p1a[:, 0:CH], w1[:, 0:DIM], hb_slice(c0),
                                 start=True, stop=True)
                nc.tensor.matmul(p1b[:, 0:CH], w1[:, 0:DIM], hb_slice(c1),
                                 start=True, stop=True)
                nc.tensor.matmul(p1a[:, CH:2 * CH], w1[:, DIM:HID], hb_slice(c0),
                                 start=True, stop=True)
                tanh(c0, p1a)
                nc.tensor.matmul(p1b[:, CH:2 * CH], w1[:, DIM:HID], hb_slice(c1),
                                 start=True, stop=True)
                tanh(c1, p1b)

            def back(s):
                """layer2 + h update + fp16 mirror for pair-slot s."""
                t, j = divmod(s, NPAIR)
                c0, c1 = 2 * j, 2 * j + 1
                at0 = ats.pop(c0)
                at1 = ats.pop(c1)
                w2s = w2s_tiles[t % 2]
                p2a = ps2.tile([DIM, CH], F32, tag="p2", name=f"p2a{s}")
                p2b = ps2.tile([DIM, CH], F32, tag="p2", name=f"p2b{s}")
                nc.tensor.matmul(p2a[:], w2s[:, 0:DIM], at0[:, 0:CH],
                                 start=True, stop=False, skip_group_check=True)
                nc.tensor.matmul(p2b[:], w2s[:, 0:DIM], at1[:, 0:CH],
                                 start=True, stop=False, skip_group_check=True)
                nc.tensor.matmul(p2a[:], w2s[:, DIM:HID], at0[:, CH:2 * CH],
                                 start=False, stop=True, skip_group_check=True)
                nc.tensor.matmul(p2b[:], w2s[:, DIM:HID], at1[:, CH:2 * CH],
                                 start=False, stop=True, skip_group_check=True)
                h = hs[j]
                hb = hbs[j]
                last = t == steps - 1

                def upd_f32(hsl, p2c):
                    """fp32 master update: h += p2 (+ dt*b2)."""
                    if b2_zero:
                        nc.vector.tensor_add(hsl, hsl, p2c[:])
                    else:
                        nc.vector.scalar_tensor_tensor(
                            hsl, p2c[:], db2[:, t:t + 1], hsl,
                            op0=ALU.add, op1=ALU.add)

                def mirror_dve(hbsl, hsl, p2c):
                    """fp16 mirror from (old h + increment); must run before
                    upd_f32 overwrites h. PSUM input keeps it full-rate."""
                    if b2_zero:
                        nc.vector.tensor_add(hbsl, hsl, p2c[:])
                    else:
                        nc.vector.scalar_tensor_tensor(
                            hbsl, p2c[:], db2[:, t:t + 1], hsl,
                            op0=ALU.add, op1=ALU.add)

                for ci, p2c in ((0, p2a), (1, p2b)):
                    hsl = h[:, ci * CH:(ci + 1) * CH]
                    hbsl = hb[:, ci * CH:(ci + 1) * CH]
                    if j < 2 and ci == 0:
                        # chunks 0 and 2: DVE mirror (pre-update), then update
                        if not last:
                            mirror_dve(hbsl, hsl, p2c)
                        upd_f32(hsl, p2c)
                    elif j < 2:
                        # chunks 1 and 3: update, then GPSIMD chunk mirror
                        upd_f32(hsl, p2c)
                        if not last:
                            nc.gpsimd.tensor_add(hbsl, hsl, zero[:, 0:CH])
                    else:
                        # pairs 2,3: update both chunks, one GPSIMD pair mirror
                        upd_f32(hsl, p2c)
                        if ci == 1 and not last:
                            nc.gpsimd.tensor_add(hb[:], h[:], zero[:])
                if last:
                    q = nc.sync if j < 2 else nc.gpsimd
                    q.dma_start(OUT[:, j * 2 * CH:(j + 1) * 2 * CH], h[:])

            for s in range(n_slots + 1):
                if s < n_slots:
                    front(s)
                if s >= 1:
                    back(s - 1)

    nc.compile()
    return nc


def make_in_maps(inputs_dict):
    """Shard + lay out the full problem inputs into per-core input maps."""
    inputs = np.ascontiguousarray(inputs_dict["inputs"], dtype=np.float32)
    timestamps = np.asarray(inputs_dict["timestamps"], dtype=np.float32)
    W1 = np.asarray(inputs_dict["W1"], dtype=np.float32)
    b1 = np.asarray(inputs_dict["b1"], dtype=np.float32)
    W2 = np.asarray(inputs_dict["W2"], dtype=np.float32)
    b2 = np.asarray(inputs_dict["b2"], dtype=np.float32)

    steps = timestamps.shape[0] - 1
    dts = np.diff(timestamps).astype(np.float32)
    w1b = np.ascontiguousarray(W1).astype(np.float16)
    # [steps, 128, 256]: per-step dt*W2, K-halves packed side by side
    w2pack = np.concatenate([W2[:DIM, :], W2[DIM:, :]], axis=1)  # [128, 256]
    w2s = (dts[:, None, None] * w2pack[None, :, :]).astype(np.float16)
    w2s = np.ascontiguousarray(w2s)
    b1p = np.ascontiguousarray(np.stack([b1[:DIM], b1[DIM:]], axis=1))
    db2 = np.ascontiguousarray(b2[:, None] * dts[None, :]).astype(np.float32)

    in_maps = []
    for i in range(N_CORES):
        shard = inputs[i * ROWS:(i + 1) * ROWS, :]
        in_maps.append({
            "hT": np.ascontiguousarray(shard.T), "W1B": w1b, "W2S": w2s,
            "B1P": b1p, "DTB2": db2,
        })
    return in_maps


def kernel(inputs, timestamps, W1, b1, W2, b2):
    timestamps = np.asarray(timestamps, dtype=np.float32)
    b1 = np.asarray(b1, dtype=np.float32)
    b2 = np.asarray(b2, dtype=np.float32)

    steps = timestamps.shape[0] - 1
    b1_zero = bool(np.all(b1 == 0.0))
    b2_zero = bool(np.all(b2 == 0.0))

    key = (steps, b1_zero, b2_zero)
    if key not in _cache:
        _cache[key] = _build(steps, b1_zero, b2_zero)
    nc = _cache[key]

    in_maps = make_in_maps({
        "inputs": inputs, "timestamps": timestamps, "W1": W1, "b1": b1,
        "W2": W2, "b2": b2,
    })

    # The axon-tunneled device occasionally reports a transient
    # "unrecoverable" state right after an unclean process exit; it clears
    # after a short wait, so retry rather than fail the whole run.
    last_exc = None
    for attempt in range(3):
        try:
            res = run_bass_kernel_spmd(nc, in_maps, core_ids=list(range(N_CORES)))
            break
        except Exception as e:
            last_exc = e
            import time as _time
            _time.sleep(20 * (attempt + 1))
    else:
        raise last_exc

    out = np.empty((BATCH, DIM), dtype=np.float32)
    for i in range(N_CORES):
        out[i * ROWS:(i + 1) * ROWS, :] = res.results[i]["hT_out"].T
    return out
